# revision 17
# baseline (speedup 1.0000x reference)
"""COIL sparse-attention scoring kernel for 8 Trainium2 NeuronCores (v3).

Strategy
--------
Shard the doc axis (Bd=128) across the 8 cores (16 docs each); qry tensors are
replicated. Exploit the match sparsity: a query position can only score against
doc tokens with the SAME token id, so the full [4096 x 2048] per-core score
matrix is ~99.6% irrelevant.

Host-side index prep (cheap): prune query rows whose id is absent from the
core's doc slab, then assign rows to tiles of 128 with a FIXED per-query quota
of 16 rows per tile (merge over the 8 per-query id-sorted lists, smallest id
first, so each tile still spans a minimal id window ~31 ids). Row r of a tile
is query q = r // 16, so the final per-query sum over rows is ONE matmul with
a compile-time constant 0/1 block mask (8 memsets) -- no selector tensor DMA,
no per-supergroup matmuls/copies.

The exact-match mask folds into the matmul: ids are rank-encoded per tile as
two base-B digit one-hots scaled by ALPHA=32 and appended to the bf16 reps, so
v[r, c] = S[r, c] + 1024 * match_digits and tok = max(v, OFF) - OFF with
OFF=2048 reproduces the reference masked-max.

Per group of GROUP tiles, one of three reduce paths (knob KERNEL_PATHS):
  d: DVE reduce_max straight from PSUM f32 + tensor_scalar(max OFF, -OFF)
  v: ScalarE relu(v-OFF)->fp16 stage, DVE fp16 reduce_max
  g: ScalarE relu(v-OFF)->fp16 stage, GpSimd fp16 reduce_max
balancing PE / ACT / DVE / Pool load. CLS scores and the final 8-way max run
on host (a few thousand elements).

DMA: inputs are cut into per-group chunks spread over both HWDGE queues
(sync: doc g0, scalar: doc g1) and the SWDGE engine pairs (one per gpsimd
dma_start, rotating), ordered first-needed-first so group-0 compute starts as
soon as its ~200KB lands instead of after the full ~750KB.
"""

import math
import os
import numpy as np
import ml_dtypes

Bq, Sq, Bd, Sd, D, Dc = 8, 512, 128, 128, 32, 768
NCORES = 8
BD_PER = Bd // NCORES          # 16 docs per core
QROWS = 128 // Bq              # 16 row slots per query per tile
ALPHA = 32.0
OFF = 2.0 * ALPHA * ALPHA      # 2048: offset of a full 2-digit rank match
GROUP = int(os.environ.get("KERNEL_GROUP", "8"))
# per-group reduce path: d=DVE-from-PSUM, v=ACT relu+DVE fp16 reduce.
# String cycles if shorter than n_groups.
PATHS = os.environ.get("KERNEL_PATHS", "dvvvv")
WARMUP_MMS = int(os.environ.get("KERNEL_WARMUP_MMS", "4"))
# docs within a group are sorted by match count and padded per class of
# BD_PER/DOC_CLASSES docs (instead of all 16 to the global max)
DOC_CLASSES = int(os.environ.get("KERNEL_DOC_CLASSES", "4"))
# walrus semaphore budget: the NEFF epilogue resets every allocated semaphore
# one instruction at a time, so fewer semaphores = shorter fixed tail.
MAX_SEMS = int(os.environ.get("KERNEL_MAX_SEMS", "0"))

_CACHE = {}


def _bf16(x):
    return x.astype(ml_dtypes.bfloat16)


def _qry_row_mask(inputs):
    """[Bq, Sq] bool: rows that can contribute (attended, not CLS/SEP)."""
    mask = np.asarray(inputs["qry_attention_mask"], np.int64).copy()
    sep = mask.sum(axis=1) - 1
    mask[np.arange(Bq), sep] = 0
    mask[:, 0] = 0
    return mask.astype(bool)


def _assign_rows(qry_ids, rows_per_query):
    """Assign rows to tiles following the GLOBAL id order (so each tile
    spans a minimal id window, like a plain 128-row cut of the global sorted
    list), but spill a row to the next tile when its query's QROWS quota is
    full. Returns (nt, tiles) where tiles[t] is a list of (partition, row)
    with partition = q*QROWS + j."""
    N = sum(len(r) for r in rows_per_query)
    allr = np.concatenate(rows_per_query).astype(np.int64)
    order = np.argsort(qry_ids[allr], kind="stable")
    grank = np.empty(len(allr), np.int64)
    grank[order] = np.arange(len(allr))
    gr = {int(r): int(g) for r, g in zip(allr, grank)}
    nt = max((len(r) + QROWS - 1) // QROWS for r in rows_per_query)
    while True:
        tiles = [[] for _ in range(nt)]
        cnt = np.zeros((nt, Bq), np.int64)
        ok = True
        for q in range(Bq):
            t_prev = 0
            for r in rows_per_query[q]:
                t = max(gr[int(r)] * nt // N, t_prev)
                while t < nt and cnt[t, q] >= QROWS:
                    t += 1
                if t >= nt:
                    ok = False
                    break
                tiles[t].append((q * QROWS + cnt[t, q], int(r)))
                cnt[t, q] += 1
                t_prev = t
            if not ok:
                break
        if ok:
            return nt, tiles
        nt += 1


def _prepare(inputs):
    """Build the per-core packed operands + the compile-time geometry.

    Returns (geom, in_maps, perms): geom is hashable and fully determines the
    Bass program; in_maps is the per-core dict of dram tensors; perms[core][g]
    is the doc permutation (sorted by match count) used for group g's columns.
    """
    qry_reps = np.asarray(inputs["qry_reps"], np.float32).reshape(-1, D)
    qry_ids = np.asarray(inputs["qry_input_ids"], np.int64).reshape(-1)
    doc_reps = np.asarray(inputs["doc_reps"], np.float32)
    doc_ids = np.asarray(inputs["doc_input_ids"], np.int64)
    row_ok = _qry_row_mask(inputs).reshape(-1)

    tiles_per_core = []
    nt = 0
    for core in range(NCORES):
        sl = slice(core * BD_PER, (core + 1) * BD_PER)
        vocab = np.zeros(1000, dtype=bool)
        vocab[doc_ids[sl].reshape(-1)] = True
        rows = np.nonzero(row_ok & vocab[qry_ids])[0]
        per_q = []
        for q in range(Bq):
            rq = rows[(rows // Sq) == q]
            per_q.append(rq[np.argsort(qry_ids[rq], kind="stable")])
        nt_c, tiles = _assign_rows(qry_ids, per_q)
        nt = max(nt, nt_c)
        tiles_per_core.append(tiles)
    n_groups = (nt + GROUP - 1) // GROUP

    # per (core, tile): id set; per (core, group, doc): match count
    idsets = [[None] * nt for _ in range(NCORES)]
    maxdist = 1
    cnt_cgd = np.zeros((NCORES, n_groups, BD_PER), dtype=np.int64)
    for core in range(NCORES):
        dids2 = doc_ids[core * BD_PER : (core + 1) * BD_PER]
        tiles = tiles_per_core[core]
        for t in range(nt):
            ent = tiles[t] if t < len(tiles) else []
            if not ent:
                idsets[core][t] = np.zeros(0, np.int64)
                continue
            idset = np.unique(qry_ids[[r for _, r in ent]])
            idsets[core][t] = idset
            maxdist = max(maxdist, len(idset))
            cnt_cgd[core, t // GROUP] = np.maximum(
                cnt_cgd[core, t // GROUP], np.isin(dids2, idset).sum(axis=1)
            )
    base = max(7, math.ceil(math.sqrt(maxdist)))
    ndig = 2 * base
    kext = D + ndig

    # doc permutation (count-desc) per (core, group); class widths uniform
    # across cores per (group, class)
    perms = [
        [np.argsort(-cnt_cgd[core, g], kind="stable") for g in range(n_groups)]
        for core in range(NCORES)
    ]
    groups = []
    for g in range(n_groups):
        ntiles = min(GROUP, nt - g * GROUP)
        scnt = np.sort(cnt_cgd[:, g], axis=1)[:, ::-1]  # [cores, BD_PER] desc
        C = DOC_CLASSES
        w = BD_PER // C
        Pks = tuple(int(scnt[:, k * w].max()) for k in range(C))
        if C > 1 and Pks[0] == 0:
            Pks = (1,) + Pks[1:]  # keep at least one nonempty class
        ds = 1
        if sum(w * p for p in Pks) > 512:
            # fall back: single class, split docs across banks, no perm
            P = max(1, int(scnt[:, 0].max()))
            ds = 1
            while (BD_PER // ds) * P > 512:
                ds *= 2
            Pks = (P,)
            for core in range(NCORES):
                perms[core][g] = np.arange(BD_PER)
        groups.append((ntiles, Pks, ds))
    geom = (kext, base, nt, tuple(groups))

    def sub_width(Pks, ds):
        if ds == 1:
            w = BD_PER // len(Pks) if len(Pks) > 1 else BD_PER
            return sum(w * p for p in Pks) if len(Pks) > 1 else BD_PER * Pks[0]
        return (BD_PER // ds) * Pks[0]

    totcol = sum(
        ntiles * ds * sub_width(Pks, ds) for ntiles, Pks, ds in groups
    )

    # Combined per-group slab layout: opsT = [qry_g0 | doc_g0 | qry_g1 |
    # doc_g1 | ...] so each group's whole input is one contiguous column
    # range (one DMA chunk with few, large packets).
    gofs = []  # per-group (slab_start, qry_cols, doc_cols)
    col = 0
    for g, (ntiles, Pks, ds) in enumerate(groups):
        qc = ntiles * 128
        dc = ntiles * ds * sub_width(Pks, ds)
        gofs.append((col, qc, dc))
        col += qc + dc
    totw = col

    in_maps = []
    for core in range(NCORES):
        tiles = tiles_per_core[core]
        dreps = doc_reps[core * BD_PER : (core + 1) * BD_PER].reshape(-1, D)
        dids = doc_ids[core * BD_PER : (core + 1) * BD_PER].reshape(-1)
        dreps_bf = _bf16(dreps).astype(np.float32)
        qreps_bf = _bf16(qry_reps).astype(np.float32)

        opsT = np.zeros((kext, totw), dtype=np.float32)
        for g, (ntiles, Pks, ds) in enumerate(groups):
            C = len(Pks)
            w = BD_PER // C if ds == 1 else BD_PER // ds
            perm = perms[core][g]
            slab, qc, dc = gofs[g]
            col = slab + qc  # doc columns start after the group's qry block
            for tl in range(ntiles):
                t = g * GROUP + tl
                ent = tiles[t] if t < len(tiles) else []
                idset = idsets[core][t]
                rank_lookup = np.full(1000, -1, np.int64)
                if len(ent):
                    rank_lookup[idset] = np.arange(len(idset))
                    parts = np.array([p for p, _ in ent])
                    rr = np.array([r for _, r in ent])
                    rk = rank_lookup[qry_ids[rr]]
                    c0 = slab + tl * 128
                    opsT[:D, c0 + parts] = qreps_bf[rr].T
                    opsT[D + rk % base, c0 + parts] = ALPHA
                    opsT[D + base + rk // base, c0 + parts] = ALPHA
                    tokmask = np.isin(dids.reshape(BD_PER, Sd), idset)
                else:
                    tokmask = np.zeros((BD_PER, Sd), dtype=bool)

                def put_doc(d, cc, pmax):
                    js = np.nonzero(tokmask[d])[0]
                    assert len(js) <= pmax
                    if len(js):
                        opsT[:D, cc : cc + len(js)] = dreps_bf[d * Sd + js].T
                        rk2 = rank_lookup[dids[d * Sd + js]]
                        opsT[D + rk2 % base, cc + np.arange(len(js))] = ALPHA
                        opsT[
                            D + base + rk2 // base, cc + np.arange(len(js))
                        ] = ALPHA

                if ds == 1:
                    cc = col
                    for k in range(C):
                        for slot in range(w):
                            put_doc(perm[k * w + slot], cc, Pks[k])
                            cc += Pks[k]
                    col = cc
                else:
                    P = Pks[0]
                    for h in range(ds):
                        for dd in range(w):
                            put_doc(h * w + dd, col + (h * w + dd) * P, P)
                    col += ds * w * P
        in_maps.append({"opsT": _bf16(opsT)})
    return geom, in_maps, perms


_LDW_PATCHED = False


def _patch_ldw_opt():
    """Append extra walrus args (opt-in via env)."""
    global _LDW_PATCHED
    extra = []
    if os.environ.get("KERNEL_LDW_OPT"):
        extra.append("--enable-ldw-opt=true")
    if MAX_SEMS:
        extra.append(f"--max-sem-num={MAX_SEMS}")
    if _LDW_PATCHED or not extra:
        return
    import concourse.bass_utils as bu

    orig = bu.get_walrus_args

    def patched(*a, **k):
        return orig(*a, **k) + extra

    bu.get_walrus_args = patched
    _LDW_PATCHED = True


def _split_multi_waits(nc, mybir):
    """This container's walrus accepts only ONE sync-wait per instruction.
    Hoist extra waits into standalone EventSemaphore instructions on the same
    engine right before the offender (sequencer blocks on each in order)."""
    n = 0
    for func in nc.m.functions:
        for bb in func.blocks:
            out = []
            for inst in bb.instructions:
                si = inst.sync_info
                if si is not None and len(si.on_wait) > 1:
                    waits = list(si.on_wait)
                    for w in waits[:-1]:
                        n += 1
                        out.append(
                            mybir.InstEventSemaphore(
                                name=f"W-{inst.name}-{n}",
                                engine=inst.engine,
                                ins=[],
                                outs=[],
                                debug=inst.debug,
                                sync_info=mybir.SyncInfo(
                                    on_wait=[w], on_update=[]
                                ),
                            )
                        )
                    inst.sync_info = mybir.SyncInfo(
                        on_wait=[waits[-1]], on_update=list(si.on_update)
                    )
                out.append(inst)
            bb.instructions = out
    return n


def _build_nc(geom):
    import concourse.bass as bass
    import concourse.mybir as mybir
    import concourse.tile as tile

    kext, base, nt, groups = geom
    bf16, f16, f32 = mybir.dt.bfloat16, mybir.dt.float16, mybir.dt.float32
    nc = bass.Bass("TRN2", target_bir_lowering=False, debug=False)

    # per-group packing info; the combined slab layout puts group g's qry
    # tile columns at [slab, slab+qc) and doc columns at [slab+qc, slab+qc+dc)
    # (t0, ntiles, Pks, ds, w, Ws, per_bank, nb, slab, gcols)
    ginfo = []
    col = 0
    for g, (ntiles, Pks, ds) in enumerate(groups):
        C = len(Pks)
        w = (BD_PER // C) if ds == 1 else (BD_PER // ds)
        Ws = sum(w * p for p in Pks)
        nsubs = ntiles * ds
        per_bank = max(1, 512 // Ws) if ds == 1 else 1
        nb = (nsubs + per_bank - 1) // per_bank
        gcols = nsubs * Ws
        ginfo.append((g * GROUP, ntiles, Pks, ds, w, Ws, per_bank, nb, col, gcols))
        col += ntiles * 128 + gcols
    totw = col
    n_groups = len(ginfo)
    paths = [PATHS[g % len(PATHS)] for g in range(n_groups)]

    opsT = nc.dram_tensor("opsT", [kext, totw], bf16, kind="ExternalInput").ap()
    out = nc.dram_tensor("out", [8, 16 * nt], f16, kind="ExternalOutput").ap()

    with tile.TileContext(nc) as tc:
        with (
            tc.tile_pool(name="inp", bufs=1) as inp,
            tc.tile_pool(name="psum", bufs=2, space="PSUM") as psum,
            tc.tile_pool(name="fpsum", bufs=1, space="PSUM") as fpsum,
            tc.tile_pool(name="stage", bufs=2) as stp,
            tc.tile_pool(name="accp", bufs=1) as accp,
        ):
            # Input SBUF + DMA. SWDGE sprays every transfer's packets across
            # all 16 SDMA engines round-robin, so per-transfer priority is
            # meaningless there and packet COUNT dominates drain time: use
            # few, large transfers. Group 0's slab rides the (isolated,
            # ~50GB/s) sync HWDGE ring so compute starts before the SWDGE
            # pool drains; the rest goes as two big SWDGE slabs.
            ops_sb = inp.tile([kext, totw], bf16)
            gb = [gi[8] for gi in ginfo] + [totw]  # slab start offsets
            ng = n_groups
            nc.sync.dma_start(ops_sb[:, 0 : gb[1]], opsT[:, 0 : gb[1]])
            if ng > 1:
                mid = gb[min(3, ng)]
                nc.gpsimd.dma_start(ops_sb[:, gb[1] : mid], opsT[:, gb[1] : mid])
                if mid < totw:
                    nc.gpsimd.dma_start(ops_sb[:, mid:totw], opsT[:, mid:totw])

            negoff = accp.tile([128, 1], f32)
            nc.vector.memset(negoff[:], -OFF)
            # tiny dummy activation: pulls the Relu ACT_TABLE_LOAD into the
            # DMA head instead of stalling the first real group
            atl = accp.tile([128, 1], f16)
            nc.scalar.activation(
                atl[:], negoff[:], mybir.ActivationFunctionType.Relu,
                bias=negoff[:],
            )

            # compile-time 0/1 block mask for the final per-query sum:
            # sel01[r, q] = 1 iff r // QROWS == q, built as the band
            # 0 <= r - QROWS*q <= QROWS-1 with two affine_selects (memsets
            # at 16-aligned partition bases are rejected by the verifier)
            sel01 = accp.tile([128, Bq], f16)
            nc.vector.memset(sel01[:], 1.0)
            nc.gpsimd.affine_select(
                sel01[:], sel01[:], [[-QROWS, Bq]],
                mybir.AluOpType.is_ge, 0.0, base=0, channel_multiplier=1,
            )
            nc.gpsimd.affine_select(
                sel01[:], sel01[:], [[QROWS, Bq]],
                mybir.AluOpType.is_ge, 0.0,
                base=QROWS - 1, channel_multiplier=-1,
            )

            # PE warm-up during the DMA head (HAM clock ramp)
            if WARMUP_MMS:
                scratch = inp.tile([kext, 512], bf16)
                nc.vector.memset(scratch[:], 0.0)
                wps = psum.tile([128, 512], f32, tag="score")
                for _ in range(WARMUP_MMS):
                    nc.tensor.matmul(
                        wps[:], scratch[:, 0:128], scratch[:],
                        start=True, stop=True,
                    )

            accum = accp.tile([128, 16 * nt], f16)
            need_draw = any(p == "d" for p in paths)
            if need_draw:
                draw = accp.tile([128, 16 * nt], f32)
            if any(0 in gi[2] for gi in ginfo):
                # zero-width classes leave accum/draw cols unwritten
                if need_draw:
                    nc.vector.memset(draw[:], 0.0)
                nc.scalar.memzero(accum[:])

            for gi, (t0, ntiles, Pks, ds, w, Ws, per_bank, nb, slab, gcols) in (
                enumerate(ginfo)
            ):
                C = len(Pks)
                offk = [sum(w * p for p in Pks[:k]) for k in range(C)]
                nsubs = ntiles * ds
                dbase = slab + ntiles * 128
                ps = psum.tile([128, nb * 512], f32, tag="score")
                # matmuls: sub j -> bank j//per_bank, slot (j%per_bank)*Ws
                for j in range(nsubs):
                    tl = j // ds
                    slot = (j // per_bank) * 512 + (j % per_bank) * Ws
                    sub = dbase + j * Ws
                    nc.tensor.matmul(
                        ps[:, slot : slot + Ws],
                        ops_sb[:, slab + tl * 128 : slab + (tl + 1) * 128],
                        ops_sb[:, sub : sub + Ws],
                        start=True,
                        stop=True,
                    )

                # chunks of subs with a regular bank pattern:
                # (bank0, nbanks, subs_per_bank, sub0)
                if ds == 1:
                    nfull = nsubs // per_bank
                    rem = nsubs % per_bank
                    chunks = []
                    if nfull:
                        chunks.append((0, nfull, per_bank, 0))
                    if rem:
                        chunks.append((nfull, 1, rem, nfull * per_bank))
                else:
                    chunks = [(0, nsubs, 1, 0)]

                def flat_view(b0, nbc, sc):
                    """[p, nb, s, Ws] strided view of the chunk's PSUM."""
                    return ps[:, b0 * 512 : (b0 + nbc) * 512].rearrange(
                        "p (nb c) -> p nb c", c=512
                    )[:, :, 0 : sc * Ws].rearrange(
                        "p nb (s c) -> p nb s c", c=Ws
                    )

                c0 = t0 * 16
                subcols = 16 // ds  # accum cols per sub
                if paths[gi] == "d":
                    for b0, nbc, sc, s0 in chunks:
                        fv = flat_view(b0, nbc, sc)
                        ob = draw[
                            :, c0 + s0 * subcols : c0 + (s0 + nbc * sc) * subcols
                        ].rearrange("p (nb s c) -> p nb s c", nb=nbc, c=subcols)
                        for k in range(C):
                            if Pks[k] == 0:
                                continue
                            nc.vector.reduce_max(
                                ob[:, :, :, k * w : (k + 1) * w],
                                fv[
                                    :, :, :, offk[k] : offk[k] + w * Pks[k]
                                ].rearrange("p nb s (d t) -> p nb s d t", t=Pks[k]),
                                axis=mybir.AxisListType.X,
                            )
                    nc.vector.tensor_scalar(
                        accum[:, c0 : c0 + 16 * ntiles],
                        draw[:, c0 : c0 + 16 * ntiles],
                        OFF,
                        -OFF,
                        mybir.AluOpType.max,
                        mybir.AluOpType.add,
                    )
                else:
                    st = stp.tile([128, nsubs * Ws], f16, tag="stage")
                    for b0, nbc, sc, s0 in chunks:
                        so = st[:, s0 * Ws : (s0 + nbc * sc) * Ws].rearrange(
                            "p (nb s c) -> p nb s c", nb=nbc, c=Ws
                        )
                        nc.scalar.activation(
                            so, flat_view(b0, nbc, sc),
                            mybir.ActivationFunctionType.Relu,
                            bias=negoff[:],
                        )
                    sv = st[:].rearrange("p (a c) -> p a c", c=Ws)
                    oacc = accum[:, c0 : c0 + 16 * ntiles].rearrange(
                        "p (a c) -> p a c", c=subcols
                    )
                    for k in range(C):
                        if Pks[k] == 0:
                            continue
                        sin = sv[:, :, offk[k] : offk[k] + w * Pks[k]].rearrange(
                            "p a (d t) -> p a d t", t=Pks[k]
                        )
                        ok = oacc[:, :, k * w : (k + 1) * w]
                        nc.vector.reduce_max(ok, sin, axis=mybir.AxisListType.X)

            # per-query partition sums: matmuls with the constant block mask
            # (one per 512-col PSUM bank); out[q, t*16+d] = sum over rows of
            # query q.
            osb = accp.tile([8, 16 * nt], f16)
            nfin = (16 * nt + 511) // 512
            fin = fpsum.tile([8, nfin * 512], f32, tag="fin")
            for j in range(nfin):
                a, b = j * 512, min((j + 1) * 512, 16 * nt)
                nc.tensor.matmul(
                    fin[:, j * 512 : j * 512 + (b - a)],
                    sel01[:],
                    accum[:, a:b],
                    start=True,
                    stop=True,
                )
                if j % 2 == 0:
                    nc.vector.tensor_copy(
                        osb[:, a:b], fin[:, j * 512 : j * 512 + (b - a)]
                    )
                else:
                    nc.scalar.copy(osb[:, a:b], fin[:, j * 512 : j * 512 + (b - a)])
                # stream each finished piece out immediately (sync is idle)
                nc.sync.dma_start(out[:, a:b], osb[:, a:b])
    _split_multi_waits(nc, mybir)
    return nc


def _get_nc(geom):
    _patch_ldw_opt()
    key = (geom, GROUP, PATHS, WARMUP_MMS)
    if key not in _CACHE:
        _CACHE[key] = _build_nc(geom)
    return _CACHE[key]


def _assemble(inputs, results, nt, perms):
    toks = np.zeros((Bq, Bd), dtype=np.float32)
    for core in range(NCORES):
        osb = np.asarray(results[core]["out"], np.float32)  # [8, 16*nt]
        part = np.zeros((Bq, BD_PER), dtype=np.float32)
        for t in range(nt):
            part[:, perms[core][t // GROUP]] += osb[:, t * 16 : (t + 1) * 16]
        toks[:, core * BD_PER : (core + 1) * BD_PER] = part
    cls = np.asarray(inputs["qry_cls"], np.float32) @ np.asarray(
        inputs["doc_cls"], np.float32
    ).T
    scores = toks + cls
    return scores.max(axis=0).reshape(-1).astype(np.float32)


def _ensure_ntff_hook():
    """This container's antenv lacks axon_hooks; synthesize the module and
    register the ctypes-based NTFF profile hook so trace=True works."""
    import sys
    import types

    if "antenv.axon_hooks" in sys.modules:
        return
    mod = types.ModuleType("antenv.axon_hooks")
    state = {"hook": None}
    mod.set_axon_ntff_profile_hook = lambda h: state.__setitem__("hook", h)
    mod.get_axon_ntff_profile_hook = lambda: state["hook"]
    sys.modules["antenv.axon_hooks"] = mod
    try:
        import antenv

        antenv.axon_hooks = mod
    except ImportError:
        pass
    try:
        from trn_agent_boot.trn_boot import _ntff_profile_via_ctypes

        mod.set_axon_ntff_profile_hook(
            _ntff_profile_via_ctypes("/opt/axon/libaxon_pjrt.so")
        )
    except Exception:
        pass


def run(inputs, trace=False, **kwargs):
    """Run on the 8 NeuronCores; returns (output, BassKernelResults)."""
    from concourse.bass_utils import run_bass_kernel_spmd

    if trace:
        _ensure_ntff_hook()
    geom, in_maps, perms = _prepare(inputs)
    nc = _get_nc(geom)
    res = run_bass_kernel_spmd(
        nc, in_maps, core_ids=list(range(NCORES)), trace=trace, **kwargs
    )
    return _assemble(inputs, res.results, geom[2], perms), res


def kernel(**inputs) -> np.ndarray:
    out, _ = run(inputs)
    return out


# revision 22
# speedup vs baseline: 1.0621x; 1.0621x over previous
"""COIL sparse-attention scoring kernel for 8 Trainium2 NeuronCores (v3).

Strategy
--------
Shard the doc axis (Bd=128) across the 8 cores (16 docs each); qry tensors are
replicated. Exploit the match sparsity: a query position can only score against
doc tokens with the SAME token id, so the full [4096 x 2048] per-core score
matrix is ~99.6% irrelevant.

Host-side index prep (cheap): prune query rows whose id is absent from the
core's doc slab, then assign rows to tiles of 128 with a FIXED per-query quota
of 16 rows per tile (merge over the 8 per-query id-sorted lists, smallest id
first, so each tile still spans a minimal id window ~31 ids). Row r of a tile
is query q = r // 16, so the final per-query sum over rows is ONE matmul with
a compile-time constant 0/1 block mask (8 memsets) -- no selector tensor DMA,
no per-supergroup matmuls/copies.

The exact-match mask folds into the matmul: ids are rank-encoded per tile as
two base-B digit one-hots scaled by ALPHA=32 and appended to the bf16 reps, so
v[r, c] = S[r, c] + 1024 * match_digits and tok = max(v, OFF) - OFF with
OFF=2048 reproduces the reference masked-max.

Per group of GROUP tiles, one of three reduce paths (knob KERNEL_PATHS):
  d: DVE reduce_max straight from PSUM f32 + tensor_scalar(max OFF, -OFF)
  v: ScalarE relu(v-OFF)->fp16 stage, DVE fp16 reduce_max
  g: ScalarE relu(v-OFF)->fp16 stage, GpSimd fp16 reduce_max
balancing PE / ACT / DVE / Pool load. CLS scores and the final 8-way max run
on host (a few thousand elements).

DMA: inputs are cut into per-group chunks spread over both HWDGE queues
(sync: doc g0, scalar: doc g1) and the SWDGE engine pairs (one per gpsimd
dma_start, rotating), ordered first-needed-first so group-0 compute starts as
soon as its ~200KB lands instead of after the full ~750KB.
"""

import math
import os
import numpy as np
import ml_dtypes

Bq, Sq, Bd, Sd, D, Dc = 8, 512, 128, 128, 32, 768
NCORES = 8
BD_PER = Bd // NCORES          # 16 docs per core
QROWS = 128 // Bq              # 16 row slots per query per tile
ALPHA = 32.0
OFF = 2.0 * ALPHA * ALPHA      # 2048: offset of a full 2-digit rank match
GROUP = int(os.environ.get("KERNEL_GROUP", "8"))
# first group is small so its input slab (the compute-start gate) is small
FIRST_GROUP = int(os.environ.get("KERNEL_FIRST_GROUP", "4"))


def _group_sizes(nt):
    sizes = [min(FIRST_GROUP, nt)]
    while sum(sizes) < nt:
        sizes.append(min(GROUP, nt - sum(sizes)))
    return sizes
# per-group reduce path: d=DVE-from-PSUM, v=ACT relu+DVE fp16 reduce.
# String cycles if shorter than n_groups.
PATHS = os.environ.get("KERNEL_PATHS", "dvvvv")
WARMUP_MMS = int(os.environ.get("KERNEL_WARMUP_MMS", "4"))
# docs within a group are sorted by match count and padded per class of
# BD_PER/DOC_CLASSES docs (instead of all 16 to the global max)
DOC_CLASSES = int(os.environ.get("KERNEL_DOC_CLASSES", "4"))
# walrus semaphore budget: the NEFF epilogue resets every allocated semaphore
# one instruction at a time, so fewer semaphores = shorter fixed tail.
MAX_SEMS = int(os.environ.get("KERNEL_MAX_SEMS", "0"))

_CACHE = {}


def _bf16(x):
    return x.astype(ml_dtypes.bfloat16)


def _qry_row_mask(inputs):
    """[Bq, Sq] bool: rows that can contribute (attended, not CLS/SEP)."""
    mask = np.asarray(inputs["qry_attention_mask"], np.int64).copy()
    sep = mask.sum(axis=1) - 1
    mask[np.arange(Bq), sep] = 0
    mask[:, 0] = 0
    return mask.astype(bool)


def _assign_rows(qry_ids, rows_per_query):
    """Assign rows to tiles following the GLOBAL id order (so each tile
    spans a minimal id window, like a plain 128-row cut of the global sorted
    list), but spill a row to the next tile when its query's QROWS quota is
    full. Returns (nt, tiles) where tiles[t] is a list of (partition, row)
    with partition = q*QROWS + j."""
    N = sum(len(r) for r in rows_per_query)
    allr = np.concatenate(rows_per_query).astype(np.int64)
    order = np.argsort(qry_ids[allr], kind="stable")
    grank = np.empty(len(allr), np.int64)
    grank[order] = np.arange(len(allr))
    gr = {int(r): int(g) for r, g in zip(allr, grank)}
    nt = max((len(r) + QROWS - 1) // QROWS for r in rows_per_query)
    while True:
        tiles = [[] for _ in range(nt)]
        cnt = np.zeros((nt, Bq), np.int64)
        ok = True
        for q in range(Bq):
            t_prev = 0
            for r in rows_per_query[q]:
                t = max(gr[int(r)] * nt // N, t_prev)
                while t < nt and cnt[t, q] >= QROWS:
                    t += 1
                if t >= nt:
                    ok = False
                    break
                tiles[t].append((q * QROWS + cnt[t, q], int(r)))
                cnt[t, q] += 1
                t_prev = t
            if not ok:
                break
        if ok:
            return nt, tiles
        nt += 1


def _prepare(inputs):
    """Build the per-core packed operands + the compile-time geometry.

    Returns (geom, in_maps, perms): geom is hashable and fully determines the
    Bass program; in_maps is the per-core dict of dram tensors; perms[core][g]
    is the doc permutation (sorted by match count) used for group g's columns.
    """
    qry_reps = np.asarray(inputs["qry_reps"], np.float32).reshape(-1, D)
    qry_ids = np.asarray(inputs["qry_input_ids"], np.int64).reshape(-1)
    doc_reps = np.asarray(inputs["doc_reps"], np.float32)
    doc_ids = np.asarray(inputs["doc_input_ids"], np.int64)
    row_ok = _qry_row_mask(inputs).reshape(-1)

    tiles_per_core = []
    nt = 0
    for core in range(NCORES):
        sl = slice(core * BD_PER, (core + 1) * BD_PER)
        vocab = np.zeros(1000, dtype=bool)
        vocab[doc_ids[sl].reshape(-1)] = True
        rows = np.nonzero(row_ok & vocab[qry_ids])[0]
        per_q = []
        for q in range(Bq):
            rq = rows[(rows // Sq) == q]
            per_q.append(rq[np.argsort(qry_ids[rq], kind="stable")])
        nt_c, tiles = _assign_rows(qry_ids, per_q)
        nt = max(nt, nt_c)
        tiles_per_core.append(tiles)
    sizes = _group_sizes(nt)
    n_groups = len(sizes)
    tg = []  # tile -> group
    for g, s in enumerate(sizes):
        tg += [g] * s

    # per (core, tile): id set; per (core, group, doc): match count
    idsets = [[None] * nt for _ in range(NCORES)]
    maxdist = 1
    cnt_cgd = np.zeros((NCORES, n_groups, BD_PER), dtype=np.int64)
    for core in range(NCORES):
        dids2 = doc_ids[core * BD_PER : (core + 1) * BD_PER]
        tiles = tiles_per_core[core]
        for t in range(nt):
            ent = tiles[t] if t < len(tiles) else []
            if not ent:
                idsets[core][t] = np.zeros(0, np.int64)
                continue
            idset = np.unique(qry_ids[[r for _, r in ent]])
            idsets[core][t] = idset
            maxdist = max(maxdist, len(idset))
            cnt_cgd[core, tg[t]] = np.maximum(
                cnt_cgd[core, tg[t]], np.isin(dids2, idset).sum(axis=1)
            )
    base = max(7, math.ceil(math.sqrt(maxdist)))
    ndig = 2 * base
    kext = D + ndig

    # doc permutation (count-desc) per (core, group); class widths uniform
    # across cores per (group, class)
    perms = [
        [np.argsort(-cnt_cgd[core, g], kind="stable") for g in range(n_groups)]
        for core in range(NCORES)
    ]
    groups = []
    for g in range(n_groups):
        ntiles = sizes[g]
        scnt = np.sort(cnt_cgd[:, g], axis=1)[:, ::-1]  # [cores, BD_PER] desc
        C = DOC_CLASSES
        w = BD_PER // C
        Pks = tuple(int(scnt[:, k * w].max()) for k in range(C))
        if C > 1 and Pks[0] == 0:
            Pks = (1,) + Pks[1:]  # keep at least one nonempty class
        ds = 1
        if sum(w * p for p in Pks) > 512:
            # fall back: single class, split docs across banks, no perm
            P = max(1, int(scnt[:, 0].max()))
            ds = 1
            while (BD_PER // ds) * P > 512:
                ds *= 2
            Pks = (P,)
            for core in range(NCORES):
                perms[core][g] = np.arange(BD_PER)
        groups.append((ntiles, Pks, ds))
    geom = (kext, base, nt, tuple(groups))

    def sub_width(Pks, ds):
        if ds == 1:
            w = BD_PER // len(Pks) if len(Pks) > 1 else BD_PER
            return sum(w * p for p in Pks) if len(Pks) > 1 else BD_PER * Pks[0]
        return (BD_PER // ds) * Pks[0]

    totcol = sum(
        ntiles * ds * sub_width(Pks, ds) for ntiles, Pks, ds in groups
    )

    # Combined per-group slab layout: opsT = [qry_g0 | doc_g0 | qry_g1 |
    # doc_g1 | ...] so each group's whole input is one contiguous column
    # range (one DMA chunk with few, large packets).
    gofs = []  # per-group (slab_start, qry_cols, doc_cols)
    col = 0
    for g, (ntiles, Pks, ds) in enumerate(groups):
        qc = ntiles * 128
        dc = ntiles * ds * sub_width(Pks, ds)
        gofs.append((col, qc, dc))
        col += qc + dc
    totw = col

    in_maps = []
    for core in range(NCORES):
        tiles = tiles_per_core[core]
        dreps = doc_reps[core * BD_PER : (core + 1) * BD_PER].reshape(-1, D)
        dids = doc_ids[core * BD_PER : (core + 1) * BD_PER].reshape(-1)
        dreps_bf = _bf16(dreps).astype(np.float32)
        qreps_bf = _bf16(qry_reps).astype(np.float32)

        opsT = np.zeros((kext, totw), dtype=np.float32)
        for g, (ntiles, Pks, ds) in enumerate(groups):
            C = len(Pks)
            w = BD_PER // C if ds == 1 else BD_PER // ds
            perm = perms[core][g]
            slab, qc, dc = gofs[g]
            tbase = sum(groups[gg][0] for gg in range(g))
            col = slab + qc  # doc columns start after the group's qry block
            for tl in range(ntiles):
                t = tbase + tl
                ent = tiles[t] if t < len(tiles) else []
                idset = idsets[core][t]
                rank_lookup = np.full(1000, -1, np.int64)
                if len(ent):
                    rank_lookup[idset] = np.arange(len(idset))
                    parts = np.array([p for p, _ in ent])
                    rr = np.array([r for _, r in ent])
                    rk = rank_lookup[qry_ids[rr]]
                    c0 = slab + tl * 128
                    opsT[:D, c0 + parts] = qreps_bf[rr].T
                    opsT[D + rk % base, c0 + parts] = ALPHA
                    opsT[D + base + rk // base, c0 + parts] = ALPHA
                    tokmask = np.isin(dids.reshape(BD_PER, Sd), idset)
                else:
                    tokmask = np.zeros((BD_PER, Sd), dtype=bool)

                def put_doc(d, cc, pmax):
                    js = np.nonzero(tokmask[d])[0]
                    assert len(js) <= pmax
                    if len(js):
                        opsT[:D, cc : cc + len(js)] = dreps_bf[d * Sd + js].T
                        rk2 = rank_lookup[dids[d * Sd + js]]
                        opsT[D + rk2 % base, cc + np.arange(len(js))] = ALPHA
                        opsT[
                            D + base + rk2 // base, cc + np.arange(len(js))
                        ] = ALPHA

                if ds == 1:
                    cc = col
                    for k in range(C):
                        for slot in range(w):
                            put_doc(perm[k * w + slot], cc, Pks[k])
                            cc += Pks[k]
                    col = cc
                else:
                    P = Pks[0]
                    for h in range(ds):
                        for dd in range(w):
                            put_doc(h * w + dd, col + (h * w + dd) * P, P)
                    col += ds * w * P
        in_maps.append({"opsT": _bf16(opsT)})
    return geom, in_maps, perms


_LDW_PATCHED = False


def _patch_ldw_opt():
    """Append extra walrus args (opt-in via env)."""
    global _LDW_PATCHED
    extra = []
    if os.environ.get("KERNEL_LDW_OPT"):
        extra.append("--enable-ldw-opt=true")
    if MAX_SEMS:
        extra.append(f"--max-sem-num={MAX_SEMS}")
    if _LDW_PATCHED or not extra:
        return
    import concourse.bass_utils as bu

    orig = bu.get_walrus_args

    def patched(*a, **k):
        return orig(*a, **k) + extra

    bu.get_walrus_args = patched
    _LDW_PATCHED = True


def _split_multi_waits(nc, mybir):
    """This container's walrus accepts only ONE sync-wait per instruction.
    Hoist extra waits into standalone EventSemaphore instructions on the same
    engine right before the offender (sequencer blocks on each in order)."""
    n = 0
    for func in nc.m.functions:
        for bb in func.blocks:
            out = []
            for inst in bb.instructions:
                si = inst.sync_info
                if si is not None and len(si.on_wait) > 1:
                    waits = list(si.on_wait)
                    for w in waits[:-1]:
                        n += 1
                        out.append(
                            mybir.InstEventSemaphore(
                                name=f"W-{inst.name}-{n}",
                                engine=inst.engine,
                                ins=[],
                                outs=[],
                                debug=inst.debug,
                                sync_info=mybir.SyncInfo(
                                    on_wait=[w], on_update=[]
                                ),
                            )
                        )
                    inst.sync_info = mybir.SyncInfo(
                        on_wait=[waits[-1]], on_update=list(si.on_update)
                    )
                out.append(inst)
            bb.instructions = out
    return n


def _build_nc(geom):
    import concourse.bass as bass
    import concourse.mybir as mybir
    import concourse.tile as tile

    kext, base, nt, groups = geom
    bf16, f16, f32 = mybir.dt.bfloat16, mybir.dt.float16, mybir.dt.float32
    nc = bass.Bass("TRN2", target_bir_lowering=False, debug=False)

    # per-group packing info; the combined slab layout puts group g's qry
    # tile columns at [slab, slab+qc) and doc columns at [slab+qc, slab+qc+dc)
    # (t0, ntiles, Pks, ds, w, Ws, per_bank, nb, slab, gcols)
    ginfo = []
    col = 0
    for g, (ntiles, Pks, ds) in enumerate(groups):
        C = len(Pks)
        w = (BD_PER // C) if ds == 1 else (BD_PER // ds)
        Ws = sum(w * p for p in Pks)
        nsubs = ntiles * ds
        per_bank = max(1, 512 // Ws) if ds == 1 else 1
        nb = (nsubs + per_bank - 1) // per_bank
        gcols = nsubs * Ws
        t0 = sum(groups[gg][0] for gg in range(g))
        ginfo.append((t0, ntiles, Pks, ds, w, Ws, per_bank, nb, col, gcols))
        col += ntiles * 128 + gcols
    totw = col
    n_groups = len(ginfo)
    paths = [PATHS[g % len(PATHS)] for g in range(n_groups)]

    opsT = nc.dram_tensor("opsT", [kext, totw], bf16, kind="ExternalInput").ap()
    out = nc.dram_tensor("out", [8, 16 * nt], f16, kind="ExternalOutput").ap()

    with tile.TileContext(nc) as tc:
        with (
            tc.tile_pool(name="inp", bufs=1) as inp,
            tc.tile_pool(name="psum", bufs=2, space="PSUM") as psum,
            tc.tile_pool(name="fpsum", bufs=1, space="PSUM") as fpsum,
            tc.tile_pool(name="stage", bufs=2) as stp,
            tc.tile_pool(name="accp", bufs=1) as accp,
        ):
            # Input SBUF + DMA. Measured DMA behavior: each SWDGE
            # dma_start's data packets ride ~one SDMA engine pair
            # (~20-40GB/s), pairs run concurrently; the two HWDGE rings
            # (sync/scalar) share one pair at ~25GB/s/engine; packets are
            # fastest at ~2KB (>=4KB per partition halves throughput). So:
            # ~1000-col chunks, group 0 (small FIRST_GROUP) on the HWDGE
            # rings so compute starts early, later groups on their own
            # SWDGE pairs first-needed-first, tail groups back on HWDGE
            # (free after g0).
            ops_sb = inp.tile([kext, totw], bf16)
            gb = [gi[8] for gi in ginfo] + [totw]  # slab start offsets
            ng = n_groups
            chunks = []  # (start, end, queue)
            for g, gi in enumerate(ginfo):
                slab, qc = gi[8], gi[1] * 128
                dend = gb[g + 1]
                if g == 0:
                    chunks.append((slab, slab + qc, "sync"))
                    chunks.append((slab + qc, dend, "scalar"))
                elif g >= ng - 2:
                    # tail groups: HWDGE rings are idle again by then
                    chunks.append((slab, slab + qc, "sync"))
                    chunks.append((slab + qc, dend, "scalar"))
                else:
                    chunks.append((slab, slab + qc, "gpsimd"))
                    chunks.append((slab + qc, dend, "gpsimd"))
            for a, b, q in chunks:
                getattr(nc, q).dma_start(ops_sb[:, a:b], opsT[:, a:b])

            negoff = accp.tile([128, 1], f32)
            nc.vector.memset(negoff[:], -OFF)
            # tiny dummy activation: pulls the Relu ACT_TABLE_LOAD into the
            # DMA head instead of stalling the first real group
            atl = accp.tile([128, 1], f16)
            nc.scalar.activation(
                atl[:], negoff[:], mybir.ActivationFunctionType.Relu,
                bias=negoff[:],
            )

            # compile-time 0/1 block mask for the final per-query sum:
            # sel01[r, q] = 1 iff r // QROWS == q, built as the band
            # 0 <= r - QROWS*q <= QROWS-1 with two affine_selects (memsets
            # at 16-aligned partition bases are rejected by the verifier)
            sel01 = accp.tile([128, Bq], f16)
            nc.vector.memset(sel01[:], 1.0)
            nc.gpsimd.affine_select(
                sel01[:], sel01[:], [[-QROWS, Bq]],
                mybir.AluOpType.is_ge, 0.0, base=0, channel_multiplier=1,
            )
            nc.gpsimd.affine_select(
                sel01[:], sel01[:], [[QROWS, Bq]],
                mybir.AluOpType.is_ge, 0.0,
                base=QROWS - 1, channel_multiplier=-1,
            )

            # PE warm-up during the DMA head (HAM clock ramp)
            if WARMUP_MMS:
                scratch = inp.tile([kext, 512], bf16)
                nc.vector.memset(scratch[:], 0.0)
                wps = psum.tile([128, 512], f32, tag="score")
                for _ in range(WARMUP_MMS):
                    nc.tensor.matmul(
                        wps[:], scratch[:, 0:128], scratch[:],
                        start=True, stop=True,
                    )

            accum = accp.tile([128, 16 * nt], f16)
            need_draw = any(p == "d" for p in paths)
            if need_draw:
                draw = accp.tile([128, 16 * nt], f32)
            if any(0 in gi[2] for gi in ginfo):
                # zero-width classes leave accum/draw cols unwritten
                if need_draw:
                    nc.vector.memset(draw[:], 0.0)
                nc.scalar.memzero(accum[:])

            for gi, (t0, ntiles, Pks, ds, w, Ws, per_bank, nb, slab, gcols) in (
                enumerate(ginfo)
            ):
                C = len(Pks)
                offk = [sum(w * p for p in Pks[:k]) for k in range(C)]
                nsubs = ntiles * ds
                dbase = slab + ntiles * 128
                ps = psum.tile([128, nb * 512], f32, tag="score")
                # matmuls: sub j -> bank j//per_bank, slot (j%per_bank)*Ws
                for j in range(nsubs):
                    tl = j // ds
                    slot = (j // per_bank) * 512 + (j % per_bank) * Ws
                    sub = dbase + j * Ws
                    nc.tensor.matmul(
                        ps[:, slot : slot + Ws],
                        ops_sb[:, slab + tl * 128 : slab + (tl + 1) * 128],
                        ops_sb[:, sub : sub + Ws],
                        start=True,
                        stop=True,
                    )

                # chunks of subs with a regular bank pattern:
                # (bank0, nbanks, subs_per_bank, sub0)
                if ds == 1:
                    nfull = nsubs // per_bank
                    rem = nsubs % per_bank
                    chunks = []
                    if nfull:
                        chunks.append((0, nfull, per_bank, 0))
                    if rem:
                        chunks.append((nfull, 1, rem, nfull * per_bank))
                else:
                    chunks = [(0, nsubs, 1, 0)]

                def flat_view(b0, nbc, sc):
                    """[p, nb, s, Ws] strided view of the chunk's PSUM."""
                    return ps[:, b0 * 512 : (b0 + nbc) * 512].rearrange(
                        "p (nb c) -> p nb c", c=512
                    )[:, :, 0 : sc * Ws].rearrange(
                        "p nb (s c) -> p nb s c", c=Ws
                    )

                c0 = t0 * 16
                subcols = 16 // ds  # accum cols per sub
                if paths[gi] == "d":
                    for b0, nbc, sc, s0 in chunks:
                        fv = flat_view(b0, nbc, sc)
                        ob = draw[
                            :, c0 + s0 * subcols : c0 + (s0 + nbc * sc) * subcols
                        ].rearrange("p (nb s c) -> p nb s c", nb=nbc, c=subcols)
                        for k in range(C):
                            if Pks[k] == 0:
                                continue
                            nc.vector.reduce_max(
                                ob[:, :, :, k * w : (k + 1) * w],
                                fv[
                                    :, :, :, offk[k] : offk[k] + w * Pks[k]
                                ].rearrange("p nb s (d t) -> p nb s d t", t=Pks[k]),
                                axis=mybir.AxisListType.X,
                            )
                    nc.vector.tensor_scalar(
                        accum[:, c0 : c0 + 16 * ntiles],
                        draw[:, c0 : c0 + 16 * ntiles],
                        OFF,
                        -OFF,
                        mybir.AluOpType.max,
                        mybir.AluOpType.add,
                    )
                else:
                    st = stp.tile([128, nsubs * Ws], f16, tag="stage")
                    for b0, nbc, sc, s0 in chunks:
                        so = st[:, s0 * Ws : (s0 + nbc * sc) * Ws].rearrange(
                            "p (nb s c) -> p nb s c", nb=nbc, c=Ws
                        )
                        nc.scalar.activation(
                            so, flat_view(b0, nbc, sc),
                            mybir.ActivationFunctionType.Relu,
                            bias=negoff[:],
                        )
                    sv = st[:].rearrange("p (a c) -> p a c", c=Ws)
                    oacc = accum[:, c0 : c0 + 16 * ntiles].rearrange(
                        "p (a c) -> p a c", c=subcols
                    )
                    for k in range(C):
                        if Pks[k] == 0:
                            continue
                        sin = sv[:, :, offk[k] : offk[k] + w * Pks[k]].rearrange(
                            "p a (d t) -> p a d t", t=Pks[k]
                        )
                        ok = oacc[:, :, k * w : (k + 1) * w]
                        nc.vector.reduce_max(ok, sin, axis=mybir.AxisListType.X)

            # per-query partition sums: matmuls with the constant block mask
            # (one per 512-col PSUM bank); out[q, t*16+d] = sum over rows of
            # query q.
            osb = accp.tile([8, 16 * nt], f16)
            nfin = (16 * nt + 511) // 512
            fin = fpsum.tile([8, nfin * 512], f32, tag="fin")
            for j in range(nfin):
                a, b = j * 512, min((j + 1) * 512, 16 * nt)
                nc.tensor.matmul(
                    fin[:, j * 512 : j * 512 + (b - a)],
                    sel01[:],
                    accum[:, a:b],
                    start=True,
                    stop=True,
                )
                if j % 2 == 0:
                    nc.vector.tensor_copy(
                        osb[:, a:b], fin[:, j * 512 : j * 512 + (b - a)]
                    )
                else:
                    nc.scalar.copy(osb[:, a:b], fin[:, j * 512 : j * 512 + (b - a)])
                # stream each finished piece out immediately (sync is idle)
                nc.sync.dma_start(out[:, a:b], osb[:, a:b])
    _split_multi_waits(nc, mybir)
    return nc


def _get_nc(geom):
    _patch_ldw_opt()
    key = (geom, GROUP, PATHS, WARMUP_MMS)
    if key not in _CACHE:
        _CACHE[key] = _build_nc(geom)
    return _CACHE[key]


def _assemble(inputs, results, nt, perms):
    sizes = _group_sizes(nt)
    tg = []
    for g, s in enumerate(sizes):
        tg += [g] * s
    toks = np.zeros((Bq, Bd), dtype=np.float32)
    for core in range(NCORES):
        osb = np.asarray(results[core]["out"], np.float32)  # [8, 16*nt]
        part = np.zeros((Bq, BD_PER), dtype=np.float32)
        for t in range(nt):
            part[:, perms[core][tg[t]]] += osb[:, t * 16 : (t + 1) * 16]
        toks[:, core * BD_PER : (core + 1) * BD_PER] = part
    cls = np.asarray(inputs["qry_cls"], np.float32) @ np.asarray(
        inputs["doc_cls"], np.float32
    ).T
    scores = toks + cls
    return scores.max(axis=0).reshape(-1).astype(np.float32)


def _ensure_ntff_hook():
    """This container's antenv lacks axon_hooks; synthesize the module and
    register the ctypes-based NTFF profile hook so trace=True works."""
    import sys
    import types

    if "antenv.axon_hooks" in sys.modules:
        return
    mod = types.ModuleType("antenv.axon_hooks")
    state = {"hook": None}
    mod.set_axon_ntff_profile_hook = lambda h: state.__setitem__("hook", h)
    mod.get_axon_ntff_profile_hook = lambda: state["hook"]
    sys.modules["antenv.axon_hooks"] = mod
    try:
        import antenv

        antenv.axon_hooks = mod
    except ImportError:
        pass
    try:
        from trn_agent_boot.trn_boot import _ntff_profile_via_ctypes

        mod.set_axon_ntff_profile_hook(
            _ntff_profile_via_ctypes("/opt/axon/libaxon_pjrt.so")
        )
    except Exception:
        pass


def run(inputs, trace=False, **kwargs):
    """Run on the 8 NeuronCores; returns (output, BassKernelResults)."""
    from concourse.bass_utils import run_bass_kernel_spmd

    if trace:
        _ensure_ntff_hook()
    geom, in_maps, perms = _prepare(inputs)
    nc = _get_nc(geom)
    res = run_bass_kernel_spmd(
        nc, in_maps, core_ids=list(range(NCORES)), trace=trace, **kwargs
    )
    return _assemble(inputs, res.results, geom[2], perms), res


def kernel(**inputs) -> np.ndarray:
    out, _ = run(inputs)
    return out
